# revision 42
# baseline (speedup 1.0000x reference)
"""Causal self-attention (B=4, S=2048, D=768, H=12) on 8 TRN2 NeuronCores.

Sharding: core = (batch b in 0..3) x (head-group hg in 0..1, 6 heads each).
Host pre-transposes x -> xT per batch (strip-major layout for wide DMA
descriptors), slices w_qkv columns / w_proj rows per head-group, and converts
all matmul operands to bf16.  Each core computes its 6 heads end-to-end and a
partial projection output [S, D]; the host sums the two head-group partials
per batch and adds b_proj plus the (attention-invariant) v-bias term
b_v @ w_proj.

Device layouts (per core):
  xT   [128, 4(strip), 6(dtile), 512] bf16 (d on partitions)
  w    [128, 3(chunk q|k|v), 6(dtile), 384] bf16
  qkT  [768(qk cols), S] bf16: tile hp (0-2) = qT of head pair hp (head0 on
       partitions 0-63, head1 on 64-127), tile 3+hp = kT of the pair.
  v    natural [S, 6, 65] bf16; col 64 of each head block is 1.0 -> the
       attn @ [v|1] matmul also emits the softmax denominator row.
  scores computed TRANSPOSED: sT[kpos, qpos] = k . q  (lhsT=kT, rhs=qT;
       bf16; the head pair's matmuls are emitted h-interleaved so the two
       K=64 row-tiles (partitions 0-63 / 64-127) run concurrently in the PE).
  exp on ScalarE over [128, 2, 512] two-PSUM-bank chunks directly from PSUM
       (scale folded into the activation), bf16 out; causal masking is a
       bf16 triangle multiply on the diagonal 128-blocks only (DVE).
  yT   [128 (pair y-dims), S] bf16 per pair -> proj lhsT directly.

The emission interleaves next-strip qkv/v matmuls and previous-strip proj
matmuls between attention chunks ("filler"), keeping the PE dense while the
ScalarE works through the exps.
"""

import numpy as np
from collections import deque
from contextlib import ExitStack

import concourse.bacc as bacc
import concourse.mybir as mybir
from concourse.tile import TileContext

F32 = mybir.dt.float32
BF16 = mybir.dt.bfloat16

D = 768
NCORES = 8
SCALE = 0.125  # 1/sqrt(64)


def build_program(S=2048):
    NS = S // 512   # q strips
    NT = S // 128   # s tiles
    DT = D // 128   # d tiles (contraction)

    nc = bacc.Bacc()

    xT = nc.dram_tensor("xT_s", [NS, 128, DT, 512], BF16, kind="ExternalInput")
    wqkv = nc.dram_tensor("wqkv_s", [3, 128, DT, 384], BF16,
                          kind="ExternalInput")
    bqk = nc.dram_tensor("bqk_s", [128, 6], F32, kind="ExternalInput")
    wproj = nc.dram_tensor("wproj_s", [128, 3, D], BF16, kind="ExternalInput")
    out = nc.dram_tensor("out_s", [S, D], F32, kind="ExternalOutput")

    with TileContext(nc) as tc, ExitStack() as ctx:
        persist = ctx.enter_context(tc.tile_pool(name="persist", bufs=1))

        qkT = [persist.tile([128, S], BF16, tag=f"qkT{i}", name=f"qkT{i}")
               for i in range(6)]
        v_sb = [persist.tile([128, 6, 65], BF16, tag=f"v{i}", name=f"v{i}")
                for i in range(NT)]
        yT = [persist.tile([128, S], BF16, tag=f"yT{i}", name=f"yT{i}")
              for i in range(3)]
        wp = persist.tile([128, 3, D], BF16, tag="wp", name="wp")
        bqk_sb = persist.tile([128, 6], F32, tag="bqk", name="bqk_sb")
        # tri2[p, h, c] = 1.0 if c - p >= 0 else 0.0 -- the causal keep-mask
        # for the [128, 128] diagonal block, replicated for both heads.
        trif = persist.tile([128, 128], F32, tag="trif", name="trif")
        tri2 = persist.tile([128, 2, 128], BF16, tag="tri", name="tri2")

        nc.vector.memset(trif[:], 1.0)
        nc.gpsimd.affine_select(
            out=trif[:], in_=trif[:],
            compare_op=mybir.AluOpType.is_ge, fill=0.0, base=0,
            pattern=[[1, 128]], channel_multiplier=-1)
        for u in range(2):
            nc.vector.tensor_copy(tri2[:, u, :], trif[:])

        nc.sync.dma_start(out=bqk_sb[:], in_=bqk[:])

        xw_pool = ctx.enter_context(tc.tile_pool(name="xw", bufs=1))
        ps = ctx.enter_context(tc.tile_pool(name="ps", bufs=1, space="PSUM"))
        expp = ctx.enter_context(tc.tile_pool(name="expp", bufs=5))
        rcp = ctx.enter_context(tc.tile_pool(name="rcp", bufs=2))
        outp = ctx.enter_context(tc.tile_pool(name="outp", bufs=4))

        xT_sb = xw_pool.tile([128, NS, DT, 512], BF16, tag="xT", name="xTs")
        w_sb = xw_pool.tile([128, 3, DT, 384], BF16, tag="w", name="ws")
        warm = xw_pool.tile([1, 256], BF16, tag="warm", name="warm")

        def pe_touch(ap):
            # Tiny matmul that makes the PE wait on this tile's producer
            # once, so later real matmuls carry at most ONE sync wait each.
            t = ps.tile([1, 1], F32, tag="mm", bufs=2, name="touch")
            nc.tensor.matmul(t[:], ap, ap, start=True, stop=True)

        nc.vector.memset(warm[:], 1.0)
        # Dummy exp primes the ACT function-table load (~2.7us) during the
        # DMA prologue instead of stalling the first attention chunk.
        actp = xw_pool.tile([1, 64], F32, tag="actp", name="actp")
        nc.scalar.activation(actp[:], warm[0:1, 0:64],
                             mybir.ActivationFunctionType.Exp, scale=1.0)
        # Warm-up stream: one accumulation group of dependency-free matmuls
        # keeps the PE HAM activity monitor busy during the input-DMA
        # prologue so the first real matmuls run at 2.4 GHz.
        wt = ps.tile([1, 256], F32, tag="mm", bufs=2, name="warm_ps")
        NWARM = 16
        for i in range(NWARM):
            nc.tensor.matmul(wt[:], warm[0:1, 0:1], warm[0:1, 0:256],
                             start=(i == 0), stop=(i == NWARM - 1))

        # input DMA triggers cost ~0.7us each on their issuing queue; spread
        # them across queues so the transfers all start early (the scalar
        # queue is busy with the ACT table-load prime, so w goes on gpsimd)
        nc.sync.dma_start(out=xT_sb[:, 0, :, :], in_=xT[0, :, :, :])
        pe_touch(xT_sb[0:1, 0, 0, 0:1])
        for ch in range(3):
            if ch > 0:
                # chain w chunks in need-order (see xT note below): q cols
                # are needed first, then k, then v
                nc.gpsimd.tensor_copy(w_sb[0:1, ch, 0, 0:2],
                                      w_sb[0:1, ch - 1, 0, 0:2])
            nc.gpsimd.dma_start(out=w_sb[:, ch, :, :], in_=wqkv[ch, :, :, :])
            pe_touch(w_sb[0:1, ch, 0, 0:1])
        # The DMA engines fair-share bandwidth across all in-flight
        # transfers, which would starve the immediately-needed strip-0/w
        # transfers.  Chain the later strips behind the earlier ones with a
        # tiny dummy copy (WAW on the DMA's destination) so each transfer
        # runs at full bandwidth in need-order.  No touches for these:
        # their consumers run tens of microseconds after the DMAs land.
        prev_gate = xT_sb[0:1, 0, 0, 0:2]
        for ns2 in range(1, NS):
            nc.gpsimd.tensor_copy(xT_sb[0:1, ns2, 0, 0:2], prev_gate)
            nc.gpsimd.dma_start(out=xT_sb[:, ns2, :, :], in_=xT[ns2, :, :, :])
            prev_gate = xT_sb[0:1, ns2, 0, 0:2]
        nc.gpsimd.tensor_copy(wp[0:1, 0, 0:2], prev_gate)
        nc.gpsimd.dma_start(out=wp[:], in_=wproj[:])

        for st in range(NT):
            nc.vector.memset(v_sb[st][:, :, 64:65], 1.0)

        def w_ap(ct, i):
            # [128, 128] column block ct (0-2 q pairs, 3-5 k pairs) of dtile i
            ch, o = ct // 3, 128 * (ct % 3)
            return w_sb[:, ch, i, o:o + 128]

        def x_strip(ns, i):
            return xT_sb[:, ns, i, :]

        def x_tile(st, i):
            o = 128 * (st % 4)
            return xT_sb[:, st // 4, i, o:o + 128]

        # ---- phase work units (emitted interleaved) ----
        def p1_unit(ns, ct):
            # qkT[128ct..][strip ns] = (wqkv[:, qk cols].T @ xT) + bias
            psu = ps.tile([128, 512], F32, tag="mm", bufs=2, name="ps_qk")
            for i in range(DT):
                nc.tensor.matmul(psu[:], w_ap(ct, i), x_strip(ns, i),
                                 start=(i == 0), stop=(i == DT - 1))
            nc.vector.tensor_scalar_add(
                qkT[ct][:, 512 * ns:512 * ns + 512], psu[:],
                bqk_sb[:, ct:ct + 1])

        def p2_unit(st):
            # v natural for s-tile st (no bias: host folds b_v @ w_proj)
            psu = ps.tile([128, 384], F32, tag="mm", bufs=2, name="ps_v")
            for i in range(DT):
                nc.tensor.matmul(psu[:], x_tile(st, i), w_sb[:, 2, i, :],
                                 start=(i == 0), stop=(i == DT - 1))
            nc.vector.tensor_copy(
                v_sb[st][:, :, 0:64],
                psu[:].rearrange("p (h e) -> p h e", h=6))

        def p4_unit(st):
            # partial proj for s-tile st
            pa = ps.tile([128, 512], F32, tag="mm", bufs=2, name="pa")
            for yt in range(3):
                nc.tensor.matmul(
                    pa[:], yT[yt][:, 128 * st:128 * st + 128],
                    wp[:, yt, 0:512], start=(yt == 0), stop=(yt == 2))
            pb = ps.tile([128, 256], F32, tag="mm", bufs=2, name="pb")
            for yt in range(3):
                nc.tensor.matmul(
                    pb[:], yT[yt][:, 128 * st:128 * st + 128],
                    wp[:, yt, 512:768], start=(yt == 0), stop=(yt == 2))
            ot = outp.tile([128, D], F32, tag="ot", name="ot")
            # ScalarE copies: keeps the PSUM-freeing reads off the busier
            # DVE queue (ScalarE has headroom and sits closer to PSUM)
            nc.scalar.copy(ot[:, 0:512], pa[:])
            nc.scalar.copy(ot[:, 512:768], pb[:])
            nc.sync.dma_start(out=out[128 * st:128 * st + 128, :], in_=ot[:])

        pre_q = deque()   # next strip's qkv/v units (due before that strip)
        opt_q = deque()   # proj units (any time after their strip)

        def drain(n, pre_only=False):
            for _ in range(n):
                if pre_q:
                    pre_q.popleft()()
                elif opt_q and not pre_only:
                    opt_q.popleft()()
                else:
                    return

        def drain_prereqs():
            # strip ns+1's qkv/v units must be fully emitted before its
            # attention reads them
            while pre_q:
                pre_q.popleft()()

        # prologue: only head-pair 0's strip-0 tiles and strip-0 v are
        # needed before attention starts; the other head pairs' qkT tiles
        # emit at their hp boundary (keeps the PE fed while ACT ramps)
        p1_unit(0, 0)
        p1_unit(0, 3)
        for st in range(2 if NS > 1 else NT):
            p2_unit(st)
        if NS > 1:
            # v tiles 2-3 are only needed by the av matmuls late in the
            # first head pair; defer them into the attention filler
            pre_q.append(lambda: p2_unit(2))
            pre_q.append(lambda: p2_unit(3))
        due_hp = {1: [(0, 1), (0, 4)], 2: [(0, 2), (0, 5)]}

        # ---- attention (with filler interleaved) ----
        for ns in range(NS):
            if ns + 1 < NS:
                for ct in range(6):
                    pre_q.append(lambda a=ns + 1, b=ct: p1_unit(a, b))
                for st in range(4 * (ns + 1), min(4 * (ns + 2), NT)):
                    pre_q.append(lambda a=st: p2_unit(a))
            q0 = 512 * ns
            for hp in range(3):
                if ns == 0:
                    for (a, b) in due_hp.pop(hp, []):
                        p1_unit(a, b)
                qt = qkT[hp]
                kt = qkT[3 + hp]
                nk = 4 * (ns + 1)
                yh = [ps.tile([65, 512], F32, tag="yh", bufs=2, name="yh0"),
                      ps.tile([65, 512], F32, tag="yh", bufs=2, name="yh1")]

                def emit_yT(kb, ex2):
                    c0 = max(0, 128 * kb - q0)
                    for h in range(2):
                        nc.tensor.matmul(
                            yh[h][:, c0:512],
                            v_sb[kb][:, 2 * hp + h, :],
                            ex2[:, h, c0:512],
                            start=(kb == 0), stop=(kb == nk - 1),
                            skip_group_check=True)

                avq = deque()  # av lags 2 k-blocks behind exp emission so
                # the ACT latency never gates the PE
                for kb in range(nk):
                    c0 = max(0, 128 * kb - q0)
                    # one [128, 2(h), 512] PSUM tile per k-block: both heads'
                    # score matmuls become ready together, so the scheduler
                    # issues them adjacently and the K=64 pair runs
                    # concurrently in the PE row-tiles (partitions 0-63 /
                    # 64-127).
                    sc2 = ps.tile([128, 2, 512], F32, tag="sc", bufs=2,
                                  name="sc2")
                    for h in range(2):
                        p0 = 64 * h
                        nc.tensor.matmul(
                            sc2[:, h, c0:512],
                            kt[p0:p0 + 64, 128 * kb:128 * kb + 128],
                            qt[p0:p0 + 64, q0 + c0:q0 + 512],
                            start=True, stop=True)
                    ex2 = expp.tile([128, 2, 512], BF16, tag="exp",
                                    name="ex2")
                    # one exp per k-block covering both heads, cropped to
                    # the causal column range
                    nc.scalar.activation(
                        ex2[:, :, c0:512], sc2[:, :, c0:512],
                        mybir.ActivationFunctionType.Exp, scale=SCALE)
                    if 128 * kb >= q0:
                        # diagonal block: zero above-diagonal entries with a
                        # bf16 triangle multiply (both heads in one op)
                        exs = ex2[:, :, c0:c0 + 128]
                        nc.vector.tensor_mul(exs, exs, tri2[:])
                    # mid strips spend only prereq filler; proj units are
                    # reserved for the last strip (which has no other
                    # filler), where the final head pair keeps them for its
                    # norm/proj tail so the PE never goes idle there
                    if ns == 0:
                        # strip 0 has 12 queued filler units but only 12
                        # k-blocks; drain every k-block so none clump at
                        # the strip boundary
                        drain(1, pre_only=True)
                    elif ns < NS - 1:
                        if kb % 2 == 1:
                            drain(1, pre_only=True)
                    elif hp < 2 and kb % 4 == 3:
                        drain(1)
                    avq.append((kb, ex2))
                    if len(avq) > 2:
                        emit_yT(*avq.popleft())
                while avq:
                    emit_yT(*avq.popleft())

                if ns == NS - 1 and hp == 2:
                    # very end of the kernel: nothing left to hide the
                    # normalization latency, and the final proj units each
                    # need only a 128-column slice of yT -- so normalize in
                    # 128-column slices (fused multiply straight from PSUM)
                    # and the first proj unit starts ~5us earlier.
                    for cs in range(4):
                        c = 128 * cs
                        for h in range(2):
                            lrow = rcp.tile([1, 128], F32, tag="lrow",
                                            name="lrow")
                            nc.vector.tensor_copy(lrow[:],
                                                  yh[h][64:65, c:c + 128])
                            rec = rcp.tile([1, 128], F32, tag="rec",
                                           name="rec")
                            nc.vector.reciprocal_approx_fast(rec[:], lrow[:])
                            rb = rcp.tile([128, 128], F32, tag="rbs",
                                          bufs=2, name="rbs")
                            nc.gpsimd.partition_broadcast(rb[:], rec[:])
                            nc.vector.tensor_mul(
                                yT[hp][64 * h:64 * h + 64,
                                       q0 + c:q0 + c + 128],
                                yh[h][0:64, c:c + 128],
                                rb[64 * h:64 * h + 64, :])
                        drain(1)
                else:
                    # tail: copies first -- they free the yh PSUM banks for
                    # the next head pair's av accumulation; the
                    # normalization chain (reciprocal of the row-64
                    # denominators, broadcast, in-place multiply) then runs
                    # off the PE critical path.
                    lrows = []
                    for h in range(2):
                        lrow = rcp.tile([1, 512], F32, tag="lrow",
                                        name="lrow")
                        nc.vector.tensor_copy(lrow[:], yh[h][64:65, :])
                        nc.vector.tensor_copy(
                            yT[hp][64 * h:64 * h + 64, q0:q0 + 512],
                            yh[h][0:64, :])
                        lrows.append(lrow)
                    for h in range(2):
                        rec = rcp.tile([1, 512], F32, tag="rec", name="rec")
                        nc.vector.reciprocal_approx_fast(rec[:], lrows[h][:])
                        rb = rcp.tile([128, 512], F32, tag="rb", bufs=2,
                                      name="rb")
                        nc.gpsimd.partition_broadcast(rb[:], rec[:])
                        ys = yT[hp][64 * h:64 * h + 64, q0:q0 + 512]
                        nc.vector.tensor_mul(ys, ys,
                                             rb[64 * h:64 * h + 64, :])
                    drain(2, pre_only=(ns < NS - 1))
            drain_prereqs()
            for st in range(4 * ns, min(4 * ns + 4, NT)):
                opt_q.append(lambda a=st: p4_unit(a))
        drain(len(opt_q))

    nc.finalize()
    return nc


def shard_inputs(x, w_qkv, b_qkv, w_proj):
    """Host-side sharding: returns list of per-core input dicts (bf16)."""
    import ml_dtypes
    BF = ml_dtypes.bfloat16
    B, S, _ = x.shape
    NS, DT = S // 512, D // 128
    in_maps = []
    for core in range(NCORES):
        b, hg = (core // 2) % B, core % 2
        cs = slice(384 * hg, 384 * hg + 384)
        # xT strip-major: [NS, 128(p), DT(i), 512(s)]; elem = x[b][ns*512+s,
        # i*128+p]
        xT_s = np.ascontiguousarray(
            x[b].reshape(NS, 512, DT, 128).transpose(0, 3, 2, 1))
        # w chunk-major: [3(q|k|v), 128(p), DT(i), 384(c)]
        W = np.concatenate(
            [w_qkv[:, 0:768][:, cs], w_qkv[:, 768:1536][:, cs],
             w_qkv[:, 1536:2304][:, cs]], axis=1)  # [768, 1152]
        wqkv_s = np.ascontiguousarray(
            W.reshape(DT, 128, 3, 384).transpose(2, 1, 0, 3))
        bqkc = np.concatenate([b_qkv[0:768][cs], b_qkv[768:1536][cs]])
        bqk_s = np.ascontiguousarray(bqkc.reshape(6, 128).T)
        # wp: [128(p), 3(r), 768(c)]
        wproj_s = np.ascontiguousarray(
            w_proj[384 * hg:384 * hg + 384, :].reshape(3, 128, D)
            .transpose(1, 0, 2))
        in_maps.append({
            "xT_s": xT_s.astype(BF),
            "wqkv_s": wqkv_s.astype(BF),
            "bqk_s": bqk_s.astype(np.float32),
            "wproj_s": wproj_s.astype(BF),
        })
    return in_maps


_CACHED = {}


def _get_program():
    if "nc" not in _CACHED:
        _CACHED["nc"] = build_program()
    return _CACHED["nc"]


def _spot_check(outp, x, w_qkv, b_qkv, w_proj, b_proj):
    """Exact per-row reference on a few rows; returns worst relative error.
    Guards against rare transient bad compiles/executions."""
    B, S, dim = x.shape
    H, HD = 12, 64
    worst = 0.0
    for b in range(B):
        s = min(S - 1, 511 + 512 * b)
        xb = x[b].astype(np.float64)
        q = xb[s] @ w_qkv[:, 0:768] + b_qkv[0:768]
        k = xb[:s + 1] @ w_qkv[:, 768:1536] + b_qkv[768:1536]
        v = xb[:s + 1] @ w_qkv[:, 1536:2304] + b_qkv[1536:2304]
        ys = []
        for h in range(H):
            sc = (k[:, HD * h:HD * h + HD] @ q[HD * h:HD * h + HD]) * 0.125
            e = np.exp(sc - sc.max())
            ys.append((e / e.sum()) @ v[:, HD * h:HD * h + HD])
        row = np.concatenate(ys) @ w_proj + b_proj
        rel = np.abs(outp[b, s] - row).max() / max(np.abs(row).max(), 1e-6)
        worst = max(worst, rel)
    return worst


def kernel(x, w_qkv, b_qkv, w_proj, b_proj):
    import jax
    from concourse.bass_utils import run_bass_kernel_spmd

    x = np.asarray(x, dtype=np.float32)
    w_qkv = np.asarray(w_qkv, dtype=np.float32)
    b_qkv = np.asarray(b_qkv, dtype=np.float32)
    w_proj = np.asarray(w_proj, dtype=np.float32)
    b_proj = np.asarray(b_proj, dtype=np.float32)

    B, S, dim = x.shape
    in_maps = shard_inputs(x, w_qkv, b_qkv, w_proj)
    # v-bias folds out of attention (rows of attn sum to exactly 1):
    # y = attn @ (v + 1 b_v^T) = attn @ v + 1 b_v^T, so its projection is a
    # constant row added on the host along with b_proj.
    bvw = b_qkv[1536:2304] @ w_proj  # [D]
    const_row = (b_proj + bvw)[None, :]

    outp = np.empty((B, S, dim), dtype=np.float32)
    for attempt in range(3):
        nc = _get_program()
        res = run_bass_kernel_spmd(nc, in_maps, core_ids=list(range(NCORES)))
        parts = [m["out_s"] for m in res.results]
        for b in range(B):
            outp[b] = parts[2 * b] + parts[2 * b + 1] + const_row
        if _spot_check(outp, x, w_qkv, b_qkv, w_proj, b_proj) < 1.2e-2:
            break
        # transient bad build/execution: clear caches, rebuild, rerun
        _CACHED.clear()
        jax.clear_caches()
    return outp


# revision 43
# speedup vs baseline: 1.0427x; 1.0427x over previous
"""Causal self-attention (B=4, S=2048, D=768, H=12) on 8 TRN2 NeuronCores.

Sharding: core = (batch b in 0..3) x (head-group hg in 0..1, 6 heads each).
Host pre-transposes x -> xT per batch (strip-major layout for wide DMA
descriptors), slices w_qkv columns / w_proj rows per head-group, and converts
all matmul operands to bf16.  Each core computes its 6 heads end-to-end and a
partial projection output [S, D]; the host sums the two head-group partials
per batch and adds b_proj plus the (attention-invariant) v-bias term
b_v @ w_proj.

Device layouts (per core):
  xT   [128, 4(strip), 6(dtile), 512] bf16 (d on partitions)
  w    [128, 3(chunk q|k|v), 6(dtile), 384] bf16
  qkT  [768(qk cols), S] bf16: tile hp (0-2) = qT of head pair hp (head0 on
       partitions 0-63, head1 on 64-127), tile 3+hp = kT of the pair.
  v    natural [S, 6, 65] bf16; col 64 of each head block is 1.0 -> the
       attn @ [v|1] matmul also emits the softmax denominator row.
  scores computed TRANSPOSED: sT[kpos, qpos] = k . q  (lhsT=kT, rhs=qT;
       bf16; the head pair's matmuls are emitted h-interleaved so the two
       K=64 row-tiles (partitions 0-63 / 64-127) run concurrently in the PE).
  exp on ScalarE over [128, 2, 512] two-PSUM-bank chunks directly from PSUM
       (scale folded into the activation), bf16 out; causal masking is a
       bf16 triangle multiply on the diagonal 128-blocks only (DVE).
  yT   [128 (pair y-dims), S] bf16 per pair -> proj lhsT directly.

The emission interleaves next-strip qkv/v matmuls and previous-strip proj
matmuls between attention chunks ("filler"), keeping the PE dense while the
ScalarE works through the exps.
"""

import numpy as np
from collections import deque
from contextlib import ExitStack

import concourse.bacc as bacc
import concourse.mybir as mybir
from concourse.tile import TileContext

F32 = mybir.dt.float32
BF16 = mybir.dt.bfloat16

D = 768
NCORES = 8
SCALE = 0.125  # 1/sqrt(64)


def build_program(S=2048):
    NS = S // 512   # q strips
    NT = S // 128   # s tiles
    DT = D // 128   # d tiles (contraction)

    nc = bacc.Bacc()

    xT = nc.dram_tensor("xT_s", [NS, 128, DT, 512], BF16, kind="ExternalInput")
    wqkv = nc.dram_tensor("wqkv_s", [3, 128, DT, 384], BF16,
                          kind="ExternalInput")
    bqk = nc.dram_tensor("bqk_s", [128, 6], F32, kind="ExternalInput")
    wproj = nc.dram_tensor("wproj_s", [128, 3, D], BF16, kind="ExternalInput")
    out = nc.dram_tensor("out_s", [S, D], F32, kind="ExternalOutput")

    with TileContext(nc) as tc, ExitStack() as ctx:
        persist = ctx.enter_context(tc.tile_pool(name="persist", bufs=1))

        qkT = [persist.tile([128, S], BF16, tag=f"qkT{i}", name=f"qkT{i}")
               for i in range(6)]
        v_sb = [persist.tile([128, 6, 65], BF16, tag=f"v{i}", name=f"v{i}")
                for i in range(NT)]
        yT = [persist.tile([128, S], BF16, tag=f"yT{i}", name=f"yT{i}")
              for i in range(3)]
        wp = persist.tile([128, 3, D], BF16, tag="wp", name="wp")
        bqk_sb = persist.tile([128, 6], F32, tag="bqk", name="bqk_sb")
        # tri2[p, h, c] = 1.0 if c - p >= 0 else 0.0 -- the causal keep-mask
        # for the [128, 128] diagonal block, replicated for both heads.
        trif = persist.tile([128, 128], F32, tag="trif", name="trif")
        tri2 = persist.tile([128, 2, 128], BF16, tag="tri", name="tri2")

        nc.vector.memset(trif[:], 1.0)
        nc.gpsimd.affine_select(
            out=trif[:], in_=trif[:],
            compare_op=mybir.AluOpType.is_ge, fill=0.0, base=0,
            pattern=[[1, 128]], channel_multiplier=-1)
        for u in range(2):
            nc.vector.tensor_copy(tri2[:, u, :], trif[:])

        nc.sync.dma_start(out=bqk_sb[:], in_=bqk[:])

        xw_pool = ctx.enter_context(tc.tile_pool(name="xw", bufs=1))
        ps = ctx.enter_context(tc.tile_pool(name="ps", bufs=1, space="PSUM"))
        expp = ctx.enter_context(tc.tile_pool(name="expp", bufs=5))
        rcp = ctx.enter_context(tc.tile_pool(name="rcp", bufs=2))
        outp = ctx.enter_context(tc.tile_pool(name="outp", bufs=4))

        xT_sb = xw_pool.tile([128, NS, DT, 512], BF16, tag="xT", name="xTs")
        w_sb = xw_pool.tile([128, 3, DT, 384], BF16, tag="w", name="ws")
        warm = xw_pool.tile([1, 256], BF16, tag="warm", name="warm")

        def pe_touch(ap):
            # Tiny matmul that makes the PE wait on this tile's producer
            # once, so later real matmuls carry at most ONE sync wait each.
            t = ps.tile([1, 1], F32, tag="mm", bufs=2, name="touch")
            nc.tensor.matmul(t[:], ap, ap, start=True, stop=True)

        nc.vector.memset(warm[:], 1.0)
        # Dummy exp primes the ACT function-table load (~2.7us) during the
        # DMA prologue instead of stalling the first attention chunk.
        actp = xw_pool.tile([1, 64], F32, tag="actp", name="actp")
        nc.scalar.activation(actp[:], warm[0:1, 0:64],
                             mybir.ActivationFunctionType.Exp, scale=1.0)
        # Warm-up stream: one accumulation group of dependency-free matmuls
        # keeps the PE HAM activity monitor busy during the input-DMA
        # prologue so the first real matmuls run at 2.4 GHz.
        wt = ps.tile([1, 256], F32, tag="mm", bufs=2, name="warm_ps")
        NWARM = 16
        for i in range(NWARM):
            nc.tensor.matmul(wt[:], warm[0:1, 0:1], warm[0:1, 0:256],
                             start=(i == 0), stop=(i == NWARM - 1))

        # input DMA triggers cost ~0.7us each on their issuing queue; spread
        # them across queues so the transfers all start early (the scalar
        # queue is busy with the ACT table-load prime, so w goes on gpsimd)
        nc.sync.dma_start(out=xT_sb[:, 0, :, :], in_=xT[0, :, :, :])
        pe_touch(xT_sb[0:1, 0, 0, 0:1])
        for ch in range(3):
            nc.gpsimd.dma_start(out=w_sb[:, ch, :, :], in_=wqkv[ch, :, :, :])
            pe_touch(w_sb[0:1, ch, 0, 0:1])
        # The DMA engines fair-share bandwidth across all in-flight
        # transfers, which would starve the immediately-needed strip-0/w
        # transfers.  Chain the later strips behind the earlier ones with a
        # tiny dummy copy (WAW on the DMA's destination) so each transfer
        # runs at full bandwidth in need-order.  No touches for these:
        # their consumers run tens of microseconds after the DMAs land.
        prev_gate = xT_sb[0:1, 0, 0, 0:2]
        for ns2 in range(1, NS):
            nc.vector.tensor_copy(xT_sb[0:1, ns2, 0, 0:2], prev_gate)
            nc.sync.dma_start(out=xT_sb[:, ns2, :, :], in_=xT[ns2, :, :, :])
            prev_gate = xT_sb[0:1, ns2, 0, 0:2]
        nc.vector.tensor_copy(wp[0:1, 0, 0:2], prev_gate)
        nc.scalar.dma_start(out=wp[:], in_=wproj[:])

        for st in range(NT):
            nc.vector.memset(v_sb[st][:, :, 64:65], 1.0)

        def w_ap(ct, i):
            # [128, 128] column block ct (0-2 q pairs, 3-5 k pairs) of dtile i
            ch, o = ct // 3, 128 * (ct % 3)
            return w_sb[:, ch, i, o:o + 128]

        def x_strip(ns, i):
            return xT_sb[:, ns, i, :]

        def x_tile(st, i):
            o = 128 * (st % 4)
            return xT_sb[:, st // 4, i, o:o + 128]

        # ---- phase work units (emitted interleaved) ----
        def p1_unit(ns, ct):
            # qkT[128ct..][strip ns] = (wqkv[:, qk cols].T @ xT) + bias
            psu = ps.tile([128, 512], F32, tag="mm", bufs=2, name="ps_qk")
            for i in range(DT):
                nc.tensor.matmul(psu[:], w_ap(ct, i), x_strip(ns, i),
                                 start=(i == 0), stop=(i == DT - 1))
            nc.vector.tensor_scalar_add(
                qkT[ct][:, 512 * ns:512 * ns + 512], psu[:],
                bqk_sb[:, ct:ct + 1])

        def p2_unit(st):
            # v natural for s-tile st (no bias: host folds b_v @ w_proj)
            psu = ps.tile([128, 384], F32, tag="mm", bufs=2, name="ps_v")
            for i in range(DT):
                nc.tensor.matmul(psu[:], x_tile(st, i), w_sb[:, 2, i, :],
                                 start=(i == 0), stop=(i == DT - 1))
            nc.vector.tensor_copy(
                v_sb[st][:, :, 0:64],
                psu[:].rearrange("p (h e) -> p h e", h=6))

        def p4_unit(st):
            # partial proj for s-tile st
            pa = ps.tile([128, 512], F32, tag="mm", bufs=2, name="pa")
            for yt in range(3):
                nc.tensor.matmul(
                    pa[:], yT[yt][:, 128 * st:128 * st + 128],
                    wp[:, yt, 0:512], start=(yt == 0), stop=(yt == 2))
            pb = ps.tile([128, 256], F32, tag="mm", bufs=2, name="pb")
            for yt in range(3):
                nc.tensor.matmul(
                    pb[:], yT[yt][:, 128 * st:128 * st + 128],
                    wp[:, yt, 512:768], start=(yt == 0), stop=(yt == 2))
            ot = outp.tile([128, D], F32, tag="ot", name="ot")
            # ScalarE copies: keeps the PSUM-freeing reads off the busier
            # DVE queue (ScalarE has headroom and sits closer to PSUM)
            nc.scalar.copy(ot[:, 0:512], pa[:])
            nc.scalar.copy(ot[:, 512:768], pb[:])
            nc.sync.dma_start(out=out[128 * st:128 * st + 128, :], in_=ot[:])

        pre_q = deque()   # next strip's qkv/v units (due before that strip)
        opt_q = deque()   # proj units (any time after their strip)

        def drain(n, pre_only=False):
            for _ in range(n):
                if pre_q:
                    pre_q.popleft()()
                elif opt_q and not pre_only:
                    opt_q.popleft()()
                else:
                    return

        def drain_prereqs():
            # strip ns+1's qkv/v units must be fully emitted before its
            # attention reads them
            while pre_q:
                pre_q.popleft()()

        # prologue: only head-pair 0's strip-0 tiles and strip-0 v are
        # needed before attention starts; the other head pairs' qkT tiles
        # emit at their hp boundary (keeps the PE fed while ACT ramps)
        p1_unit(0, 0)
        p1_unit(0, 3)
        for st in range(2 if NS > 1 else NT):
            p2_unit(st)
        if NS > 1:
            # v tiles 2-3 are only needed by the av matmuls late in the
            # first head pair; defer them into the attention filler
            pre_q.append(lambda: p2_unit(2))
            pre_q.append(lambda: p2_unit(3))
        due_hp = {1: [(0, 1), (0, 4)], 2: [(0, 2), (0, 5)]}

        # ---- attention (with filler interleaved) ----
        for ns in range(NS):
            if ns + 1 < NS:
                for ct in range(6):
                    pre_q.append(lambda a=ns + 1, b=ct: p1_unit(a, b))
                for st in range(4 * (ns + 1), min(4 * (ns + 2), NT)):
                    pre_q.append(lambda a=st: p2_unit(a))
            q0 = 512 * ns
            for hp in range(3):
                if ns == 0:
                    for (a, b) in due_hp.pop(hp, []):
                        p1_unit(a, b)
                qt = qkT[hp]
                kt = qkT[3 + hp]
                nk = 4 * (ns + 1)
                yh = [ps.tile([65, 512], F32, tag="yh", bufs=2, name="yh0"),
                      ps.tile([65, 512], F32, tag="yh", bufs=2, name="yh1")]

                def emit_yT(kb, ex2):
                    c0 = max(0, 128 * kb - q0)
                    for h in range(2):
                        nc.tensor.matmul(
                            yh[h][:, c0:512],
                            v_sb[kb][:, 2 * hp + h, :],
                            ex2[:, h, c0:512],
                            start=(kb == 0), stop=(kb == nk - 1),
                            skip_group_check=True)

                avq = deque()  # av lags 2 k-blocks behind exp emission so
                # the ACT latency never gates the PE
                for kb in range(nk):
                    c0 = max(0, 128 * kb - q0)
                    # one [128, 2(h), 512] PSUM tile per k-block: both heads'
                    # score matmuls become ready together, so the scheduler
                    # issues them adjacently and the K=64 pair runs
                    # concurrently in the PE row-tiles (partitions 0-63 /
                    # 64-127).
                    sc2 = ps.tile([128, 2, 512], F32, tag="sc", bufs=2,
                                  name="sc2")
                    for h in range(2):
                        p0 = 64 * h
                        nc.tensor.matmul(
                            sc2[:, h, c0:512],
                            kt[p0:p0 + 64, 128 * kb:128 * kb + 128],
                            qt[p0:p0 + 64, q0 + c0:q0 + 512],
                            start=True, stop=True)
                    ex2 = expp.tile([128, 2, 512], BF16, tag="exp",
                                    name="ex2")
                    # one exp per k-block covering both heads, cropped to
                    # the causal column range
                    nc.scalar.activation(
                        ex2[:, :, c0:512], sc2[:, :, c0:512],
                        mybir.ActivationFunctionType.Exp, scale=SCALE)
                    if 128 * kb >= q0:
                        # diagonal block: zero above-diagonal entries with a
                        # bf16 triangle multiply (both heads in one op)
                        exs = ex2[:, :, c0:c0 + 128]
                        nc.vector.tensor_mul(exs, exs, tri2[:])
                    # mid strips spend only prereq filler; proj units are
                    # reserved for the last strip (which has no other
                    # filler), where the final head pair keeps them for its
                    # norm/proj tail so the PE never goes idle there
                    if ns == 0:
                        # strip 0 has 12 queued filler units but only 12
                        # k-blocks; drain every k-block so none clump at
                        # the strip boundary
                        drain(1, pre_only=True)
                    elif ns < NS - 1:
                        if kb % 2 == 1:
                            drain(1, pre_only=True)
                    elif hp < 2 and kb % 4 == 3:
                        drain(1)
                    avq.append((kb, ex2))
                    if len(avq) > 2:
                        emit_yT(*avq.popleft())
                while avq:
                    emit_yT(*avq.popleft())

                if ns == NS - 1 and hp == 2:
                    # very end of the kernel: nothing left to hide the
                    # normalization latency, and the final proj units each
                    # need only a 128-column slice of yT -- so normalize in
                    # 128-column slices (fused multiply straight from PSUM)
                    # and the first proj unit starts ~5us earlier.
                    for cs in range(4):
                        c = 128 * cs
                        for h in range(2):
                            lrow = rcp.tile([1, 128], F32, tag="lrow",
                                            name="lrow")
                            nc.vector.tensor_copy(lrow[:],
                                                  yh[h][64:65, c:c + 128])
                            rec = rcp.tile([1, 128], F32, tag="rec",
                                           name="rec")
                            nc.vector.reciprocal_approx_fast(rec[:], lrow[:])
                            rb = rcp.tile([128, 128], F32, tag="rbs",
                                          bufs=2, name="rbs")
                            nc.gpsimd.partition_broadcast(rb[:], rec[:])
                            nc.vector.tensor_mul(
                                yT[hp][64 * h:64 * h + 64,
                                       q0 + c:q0 + c + 128],
                                yh[h][0:64, c:c + 128],
                                rb[64 * h:64 * h + 64, :])
                        drain(1)
                else:
                    # tail: copies first -- they free the yh PSUM banks for
                    # the next head pair's av accumulation; the
                    # normalization chain (reciprocal of the row-64
                    # denominators, broadcast, in-place multiply) then runs
                    # off the PE critical path.
                    lrows = []
                    for h in range(2):
                        lrow = rcp.tile([1, 512], F32, tag="lrow",
                                        name="lrow")
                        nc.vector.tensor_copy(lrow[:], yh[h][64:65, :])
                        nc.vector.tensor_copy(
                            yT[hp][64 * h:64 * h + 64, q0:q0 + 512],
                            yh[h][0:64, :])
                        lrows.append(lrow)
                    for h in range(2):
                        rec = rcp.tile([1, 512], F32, tag="rec", name="rec")
                        nc.vector.reciprocal_approx_fast(rec[:], lrows[h][:])
                        rb = rcp.tile([128, 512], F32, tag="rb", bufs=2,
                                      name="rb")
                        nc.gpsimd.partition_broadcast(rb[:], rec[:])
                        ys = yT[hp][64 * h:64 * h + 64, q0:q0 + 512]
                        nc.vector.tensor_mul(ys, ys,
                                             rb[64 * h:64 * h + 64, :])
                    drain(2, pre_only=(ns < NS - 1))
            drain_prereqs()
            for st in range(4 * ns, min(4 * ns + 4, NT)):
                opt_q.append(lambda a=st: p4_unit(a))
        drain(len(opt_q))

    nc.finalize()
    return nc


def shard_inputs(x, w_qkv, b_qkv, w_proj):
    """Host-side sharding: returns list of per-core input dicts (bf16)."""
    import ml_dtypes
    BF = ml_dtypes.bfloat16
    B, S, _ = x.shape
    NS, DT = S // 512, D // 128
    in_maps = []
    for core in range(NCORES):
        b, hg = (core // 2) % B, core % 2
        cs = slice(384 * hg, 384 * hg + 384)
        # xT strip-major: [NS, 128(p), DT(i), 512(s)]; elem = x[b][ns*512+s,
        # i*128+p]
        xT_s = np.ascontiguousarray(
            x[b].reshape(NS, 512, DT, 128).transpose(0, 3, 2, 1))
        # w chunk-major: [3(q|k|v), 128(p), DT(i), 384(c)]
        W = np.concatenate(
            [w_qkv[:, 0:768][:, cs], w_qkv[:, 768:1536][:, cs],
             w_qkv[:, 1536:2304][:, cs]], axis=1)  # [768, 1152]
        wqkv_s = np.ascontiguousarray(
            W.reshape(DT, 128, 3, 384).transpose(2, 1, 0, 3))
        bqkc = np.concatenate([b_qkv[0:768][cs], b_qkv[768:1536][cs]])
        bqk_s = np.ascontiguousarray(bqkc.reshape(6, 128).T)
        # wp: [128(p), 3(r), 768(c)]
        wproj_s = np.ascontiguousarray(
            w_proj[384 * hg:384 * hg + 384, :].reshape(3, 128, D)
            .transpose(1, 0, 2))
        in_maps.append({
            "xT_s": xT_s.astype(BF),
            "wqkv_s": wqkv_s.astype(BF),
            "bqk_s": bqk_s.astype(np.float32),
            "wproj_s": wproj_s.astype(BF),
        })
    return in_maps


_CACHED = {}


def _get_program():
    if "nc" not in _CACHED:
        _CACHED["nc"] = build_program()
    return _CACHED["nc"]


def _spot_check(outp, x, w_qkv, b_qkv, w_proj, b_proj):
    """Exact per-row reference on a few rows; returns worst relative error.
    Guards against rare transient bad compiles/executions."""
    B, S, dim = x.shape
    H, HD = 12, 64
    worst = 0.0
    for b in range(B):
        s = min(S - 1, 511 + 512 * b)
        xb = x[b].astype(np.float64)
        q = xb[s] @ w_qkv[:, 0:768] + b_qkv[0:768]
        k = xb[:s + 1] @ w_qkv[:, 768:1536] + b_qkv[768:1536]
        v = xb[:s + 1] @ w_qkv[:, 1536:2304] + b_qkv[1536:2304]
        ys = []
        for h in range(H):
            sc = (k[:, HD * h:HD * h + HD] @ q[HD * h:HD * h + HD]) * 0.125
            e = np.exp(sc - sc.max())
            ys.append((e / e.sum()) @ v[:, HD * h:HD * h + HD])
        row = np.concatenate(ys) @ w_proj + b_proj
        rel = np.abs(outp[b, s] - row).max() / max(np.abs(row).max(), 1e-6)
        worst = max(worst, rel)
    return worst


def kernel(x, w_qkv, b_qkv, w_proj, b_proj):
    import jax
    from concourse.bass_utils import run_bass_kernel_spmd

    x = np.asarray(x, dtype=np.float32)
    w_qkv = np.asarray(w_qkv, dtype=np.float32)
    b_qkv = np.asarray(b_qkv, dtype=np.float32)
    w_proj = np.asarray(w_proj, dtype=np.float32)
    b_proj = np.asarray(b_proj, dtype=np.float32)

    B, S, dim = x.shape
    in_maps = shard_inputs(x, w_qkv, b_qkv, w_proj)
    # v-bias folds out of attention (rows of attn sum to exactly 1):
    # y = attn @ (v + 1 b_v^T) = attn @ v + 1 b_v^T, so its projection is a
    # constant row added on the host along with b_proj.
    bvw = b_qkv[1536:2304] @ w_proj  # [D]
    const_row = (b_proj + bvw)[None, :]

    outp = np.empty((B, S, dim), dtype=np.float32)
    for attempt in range(3):
        nc = _get_program()
        res = run_bass_kernel_spmd(nc, in_maps, core_ids=list(range(NCORES)))
        parts = [m["out_s"] for m in res.results]
        for b in range(B):
            outp[b] = parts[2 * b] + parts[2 * b + 1] + const_row
        if _spot_check(outp, x, w_qkv, b_qkv, w_proj, b_proj) < 1.2e-2:
            break
        # transient bad build/execution: clear caches, rebuild, rerun
        _CACHED.clear()
        jax.clear_caches()
    return outp
